# revision 8
# baseline (speedup 1.0000x reference)
"""Fused MoE dispatch (row gather) + SwiGLU + per-expert smooth scale +
per-row dynamic int8 quantization, SPMD across 8 NeuronCores.

Contract: kernel(**inputs) takes FULL inputs, returns (quantized [N, d] int8,
quant_scale [N] f32). Rows are sharded 8 ways; the gather source table
(`input`) and the small smooth_scale table are replicated per core.
"""

import os
import numpy as np

import concourse.bass as bass
import concourse.bacc as bacc
from concourse import mybir
from concourse.tile import TileContext
from concourse.bass_utils import run_bass_kernel_spmd

N = 16384          # dispatched rows (full)
FC1 = 4096         # gate|up concatenated
D = FC1 // 2       # 2048
E = 8              # experts
NCORES = 8
ROWS = N // NCORES  # 2048 rows per core
P = 128            # partitions per tile
NTILES = ROWS // P  # 16 tiles per core
QMAX = 127.0

_cache = {}


def _build():
    nc = bacc.Bacc("TRN2")
    x = nc.dram_tensor("x", [N, FC1], mybir.dt.float32, kind="ExternalInput")
    ss = nc.dram_tensor("ss", [E, D], mybir.dt.float32, kind="ExternalInput")
    # column t holds tile t's 128 row indices / expert ids
    sid = nc.dram_tensor("sid", [P, NTILES], mybir.dt.int32, kind="ExternalInput")
    tki = nc.dram_tensor("tki", [P, NTILES], mybir.dt.int32, kind="ExternalInput")
    ident = nc.dram_tensor("ident", [P, P], mybir.dt.float32, kind="ExternalInput")
    iota8 = nc.dram_tensor("iota8", [P, E], mybir.dt.float32, kind="ExternalInput")
    qout = nc.dram_tensor("qout", [ROWS, D], mybir.dt.int8, kind="ExternalOutput")
    sout = nc.dram_tensor("sout", [ROWS, 1], mybir.dt.float32, kind="ExternalOutput")

    with TileContext(nc) as tc:
        with tc.tile_pool(name="idx", bufs=1) as idxp, \
             tc.tile_pool(name="g", bufs=4) as gp, \
             tc.tile_pool(name="sc", bufs=1, space="PSUM") as scp, \
             tc.tile_pool(name="cst", bufs=1) as cstp, \
             tc.tile_pool(name="oh", bufs=2, space="PSUM") as ohpp, \
             tc.tile_pool(name="act", bufs=4) as actp, \
             tc.tile_pool(name="q", bufs=4) as qp, \
             tc.tile_pool(name="small", bufs=8) as smp:
            sid_t = idxp.tile([P, NTILES], mybir.dt.int32)
            tki_t = idxp.tile([P, NTILES], mybir.dt.int32)
            nc.sync.dma_start(out=sid_t[:], in_=sid[:])
            nc.sync.dma_start(out=tki_t[:], in_=tki[:])

            ident_t = cstp.tile([P, P], mybir.dt.float32)
            iota_t = cstp.tile([P, E], mybir.dt.float32)
            ss_t = cstp.tile([E, D], mybir.dt.float32)
            nc.sync.dma_start(out=ident_t[:], in_=ident[:])
            nc.sync.dma_start(out=iota_t[:], in_=iota8[:])
            nc.sync.dma_start(out=ss_t[:], in_=ss[:])
            tki_f = cstp.tile([P, NTILES], mybir.dt.float32)
            nc.vector.tensor_copy(out=tki_f[:], in_=tki_t[:])
            oh = cstp.tile([P, P], mybir.dt.float32)
            for t in range(NTILES):
                nc.vector.tensor_scalar(
                    out=oh[:, t * E:(t + 1) * E], in0=iota_t[:],
                    scalar1=tki_f[:, t:t + 1], scalar2=None,
                    op0=mybir.AluOpType.is_equal,
                )
            ohT = cstp.tile([E, NTILES * P], mybir.dt.float32)
            for t in range(NTILES):
                ohT_ps = ohpp.tile([E, P], mybir.dt.float32, tag="ohT_ps")
                nc.tensor.transpose(
                    out=ohT_ps[:], in_=oh[:, t * E:(t + 1) * E],
                    identity=ident_t[:],
                )
                nc.vector.tensor_copy(
                    out=ohT[:, t * P:(t + 1) * P], in_=ohT_ps[:],
                )

            for t in range(NTILES):
                g_t = gp.tile([P, FC1], mybir.dt.float32, tag="g")
                nc.gpsimd.indirect_dma_start(
                    out=g_t[:], out_offset=None, in_=x[:],
                    in_offset=bass.IndirectOffsetOnAxis(ap=sid_t[:, t:t + 1], axis=0),
                )
                sc_t = scp.tile([P, D], mybir.dt.float32, tag="sc")
                for c in range(D // 512):
                    nc.tensor.matmul(
                        out=sc_t[:, c * 512:(c + 1) * 512],
                        lhsT=ohT[:, t * P:(t + 1) * P],
                        rhs=ss_t[:, c * 512:(c + 1) * 512],
                        start=True, stop=True,
                    )

                act_t = actp.tile([P, D], mybir.dt.float32, tag="act")
                nc.scalar.activation(
                    out=act_t[:], in_=g_t[:, :D],
                    func=mybir.ActivationFunctionType.Silu,
                )
                nc.vector.tensor_mul(out=act_t[:], in0=act_t[:], in1=g_t[:, D:])
                nc.vector.tensor_mul(out=act_t[:], in0=act_t[:], in1=sc_t[:])

                amax = smp.tile([P, 1], mybir.dt.float32, tag="amax")
                nc.vector.tensor_reduce(
                    out=amax[:], in_=act_t[:], axis=mybir.AxisListType.X,
                    op=mybir.AluOpType.max, apply_absolute_value=True,
                )
                # qs = amax/127, +1.0 where amax == 0 (reference `where`)
                is0 = smp.tile([P, 1], mybir.dt.float32, tag="is0")
                nc.vector.tensor_scalar(
                    out=is0[:], in0=amax[:], scalar1=0.0, scalar2=None,
                    op0=mybir.AluOpType.is_equal,
                )
                qs = smp.tile([P, 1], mybir.dt.float32, tag="qs")
                nc.vector.tensor_scalar(
                    out=qs[:], in0=amax[:], scalar1=1.0 / QMAX, scalar2=None,
                    op0=mybir.AluOpType.mult,
                )
                nc.vector.tensor_add(out=qs[:], in0=qs[:], in1=is0[:])
                rqs = smp.tile([P, 1], mybir.dt.float32, tag="rqs")
                nc.vector.reciprocal(out=rqs[:], in_=qs[:])

                # q = cast_int8(act * (1/qs))  (RNE + saturation == clip/round)
                q_t = qp.tile([P, D], mybir.dt.int8, tag="q")
                nc.vector.tensor_scalar(
                    out=q_t[:], in0=act_t[:], scalar1=rqs[:, :1], scalar2=None,
                    op0=mybir.AluOpType.mult,
                )
                nc.sync.dma_start(out=qout[t * P:(t + 1) * P, :], in_=q_t[:])
                nc.sync.dma_start(out=sout[t * P:(t + 1) * P, :], in_=qs[:])
    nc.finalize()
    return nc


def _get_nc():
    if "nc" not in _cache:
        _cache["nc"] = _build()
    return _cache["nc"]


def kernel(input, smooth_scale, sorted_token_ids, topk_indices,
           fc1_intermediate_size):
    assert int(fc1_intermediate_size) == FC1
    x = np.ascontiguousarray(np.asarray(input, dtype=np.float32).reshape(N, FC1))
    ss = np.ascontiguousarray(np.asarray(smooth_scale, dtype=np.float32))
    sid = np.asarray(sorted_token_ids, dtype=np.int32).reshape(NCORES, NTILES, P)
    tki = np.asarray(topk_indices, dtype=np.int32).reshape(NCORES, NTILES, P)

    nc = _get_nc()
    in_maps = []
    for c in range(NCORES):
        in_maps.append({
            "x": x,
            "ss": ss,
            "sid": np.ascontiguousarray(sid[c].T),   # [P, NTILES]
            "tki": np.ascontiguousarray(tki[c].T),
            "ident": np.eye(P, dtype=np.float32),
            "iota8": np.tile(np.arange(E, dtype=np.float32), (P, 1)),
        })
    trace = bool(int(os.environ.get("BASSK_TRACE", "0")))
    res = run_bass_kernel_spmd(
        nc, in_maps, core_ids=list(range(NCORES)), trace=trace,
    )
    _cache["last_result"] = res
    q = np.concatenate([r["qout"] for r in res.results], axis=0)
    s = np.concatenate([r["sout"] for r in res.results], axis=0).reshape(-1)
    return q.astype(np.int8), s.astype(np.float32)
